# revision 1
# baseline (speedup 1.0000x reference)
"""Trainium2 Bass kernel for nn_CgpHmmLayer (HMM forward-algorithm log-likelihood).

Problem: batch=64 one-hot sequences [64, 4096, 32], softmax-parameterized HMM
with 128 states; output loglik [64].

Sharding: data-parallel over batch across 8 NeuronCores (8 sequences/core),
A/B/I replicated. No collectives needed.

Device algorithm (per core, states on partitions):
  A   = softmax(A_logits, rows)            (bf16 stationary for the scan matmul)
  expB = exp(B_logits);  r32[s] = 32 / sum_a expB[a,s]
  Ehat^T[s, (t,b)] = r32[s] * (expB^T @ X^T)   (bf16, 8MB in SBUF)
      -- the emission einsum, with a constant 32x rescale folded in so the
         running forward variable stays O(1) and renorms can be sparse.
  alpha_0 = expI * Ehat_0
  step t:  alpha = (A^T @ alpha) * Ehat_t      (PE matmul -> DVE multiply)
  every 64 steps (tick t0=64k+48): s~_b = colsum(alpha_t0) via ones-matmul;
      1/s~ broadcast via rank-1 matmul and folded into the Ehat slice consumed
      at t0+16 (off the critical serial chain); ln(s~) accumulated.
  loglik = ln(colsum(alpha_T)) + sum ln(s~) - T*ln(32) - ln(sum expI)

The one-hot input is shipped pre-transposed as bf16 [32, T*8] per core (pure
layout marshalling; 0/1 are exact in bf16) so the emission matmul's contraction
dim (alphabet) lands on partitions without any on-device transpose.
"""
import math
from contextlib import ExitStack

import numpy as np

B, T, ALPH, S = 64, 4096, 32, 128
NC = 8
NB = B // NC  # sequences per core

REN_PERIOD = 64
REN_TICK = 48    # renorm ticks t0 = k*64 + 48
REN_DELTA = 16   # applied at t0 + 16
ECHUNK = 512

_COMPILED = None


def _kernel_body(tc, xT, aL, bL, iL, out, t_len):
    import concourse.bass as bass
    from concourse import mybir

    nc = tc.nc
    f32 = mybir.dt.float32
    bf16 = mybir.dt.bfloat16
    AX = mybir.AxisListType
    OP = mybir.AluOpType
    AF = mybir.ActivationFunctionType

    n_echunks = (t_len * NB) // ECHUNK

    with ExitStack() as ctx:
        singles = ctx.enter_context(tc.tile_pool(name="singles", bufs=1))
        epsum = ctx.enter_context(tc.tile_pool(name="epsum", bufs=2, space="PSUM"))
        spsum = ctx.enter_context(tc.tile_pool(name="spsum", bufs=2, space="PSUM"))
        rpsum = ctx.enter_context(tc.tile_pool(name="rpsum", bufs=1, space="PSUM"))
        bpsum = ctx.enter_context(tc.tile_pool(name="bpsum", bufs=1, space="PSUM"))
        apool = ctx.enter_context(tc.tile_pool(name="apool", bufs=3))
        rpool = ctx.enter_context(tc.tile_pool(name="rpool", bufs=2))

        # ---------------- parameter prep ----------------
        aL_sb = singles.tile([S, S], f32)
        nc.sync.dma_start(aL_sb[:], aL)
        bN_sb = singles.tile([ALPH, S], f32)
        nc.sync.dma_start(bN_sb[:], bL)
        iL_sb = singles.tile([S, 1], f32)
        nc.sync.dma_start(
            iL_sb[:], bass.AP(tensor=iL.tensor, offset=0, ap=[[1, S], [S, 1]])
        )

        # A = softmax(rows of A_logits), stored bf16 (scan stationary operand)
        rowmax = singles.tile([S, 1], f32)
        nc.vector.tensor_reduce(rowmax[:], aL_sb[:], axis=AX.X, op=OP.max)
        negmax = singles.tile([S, 1], f32)
        nc.vector.tensor_scalar_mul(negmax[:], rowmax[:], -1.0)
        expA = singles.tile([S, S], f32)
        nc.scalar.activation(expA[:], aL_sb[:], AF.Exp, bias=negmax[:], scale=1.0)
        rowsum = singles.tile([S, 1], f32)
        nc.vector.tensor_reduce(rowsum[:], expA[:], axis=AX.X, op=OP.add)
        rrow = singles.tile([S, 1], f32)
        nc.vector.reciprocal(rrow[:], rowsum[:])
        A_sb = singles.tile([S, S], bf16)
        nc.vector.tensor_scalar_mul(A_sb[:], expA[:], rrow[:])

        # expB (column softmax handled via r32 scale folded into Ehat).
        # B_logits ~ N(0,1) so exp() without max-subtraction is safe.
        expB = singles.tile([ALPH, S], bf16)
        nc.scalar.activation(expB[:], bN_sb[:], AF.Exp)
        ones32 = singles.tile([ALPH, 1], bf16)
        nc.vector.memset(ones32[:], 1.0)
        bsum_ps = bpsum.tile([S, 1], f32, tag="bsum")
        nc.tensor.matmul(bsum_ps[:], expB[:], ones32[:], start=True, stop=True)
        r32 = singles.tile([S, 1], f32)
        nc.vector.reciprocal(r32[:], bsum_ps[:])
        nc.vector.tensor_scalar_mul(r32[:], r32[:], 32.0)

        # expI (fp32 for the alpha_0 scale; bf16 for the sum matmul)
        expI = singles.tile([S, 1], f32)
        nc.scalar.activation(expI[:], iL_sb[:], AF.Exp)
        expI_h = singles.tile([S, 1], bf16)
        nc.vector.tensor_copy(expI_h[:], expI[:])

        ones_col = singles.tile([S, 1], bf16)
        nc.vector.memset(ones_col[:], 1.0)
        ones_row = singles.tile([1, S], bf16)
        nc.vector.memset(ones_row[:], 1.0)

        sumi_ps = bpsum.tile([1, 1], f32, tag="bsum")
        nc.tensor.matmul(sumi_ps[:], ones_col[:], expI_h[:], start=True, stop=True)
        ln_sumi = singles.tile([1, 1], f32)
        nc.scalar.activation(ln_sumi[:], sumi_ps[:], AF.Ln)

        # ---------------- emission precompute ----------------
        xT_sb = singles.tile([ALPH, t_len * NB], bf16)
        for i in range(max(1, n_echunks // 8)):
            w = 8 * ECHUNK
            lo, hi = i * w, min((i + 1) * w, t_len * NB)
            nc.sync.dma_start(xT_sb[:, lo:hi], xT[:, lo:hi])

        ehat = singles.tile([S, t_len * NB], bf16)
        ehat_v = ehat[:].rearrange("s (t nb) -> s t nb", nb=NB)
        for c in range(n_echunks):
            lo, hi = c * ECHUNK, (c + 1) * ECHUNK
            e_ps = epsum.tile([S, ECHUNK], f32, tag="eps")
            nc.tensor.matmul(e_ps[:], expB[:], xT_sb[:, lo:hi], start=True, stop=True)
            nc.vector.tensor_scalar_mul(ehat[:, lo:hi], e_ps[:], r32[:])

        # ---------------- the scan ----------------
        acc = singles.tile([1, NB], f32)
        nc.vector.memset(acc[:], 0.0)

        alpha = apool.tile([S, NB], bf16, tag="alpha")
        nc.vector.tensor_scalar_mul(alpha[:], ehat_v[:, 0, :], expI[:])

        n_ren = 0
        pending = {}  # t_app -> bcast psum tile
        for t in range(1, t_len):
            mm_ps = spsum.tile([S, NB], f32, tag="mm")
            nc.tensor.matmul(mm_ps[:], A_sb[:], alpha[:], start=True, stop=True)

            if t in pending:
                e_in = pending.pop(t)
            else:
                e_in = ehat_v[:, t, :]
            alpha_new = apool.tile([S, NB], bf16, tag="alpha")
            nc.vector.tensor_mul(alpha_new[:], mm_ps[:], e_in)
            alpha = alpha_new

            if t % REN_PERIOD == REN_TICK and t + REN_DELTA < t_len:
                # column sums of alpha via ones-matmul (cheap PE visit)
                s_ps = rpsum.tile([1, NB], f32, tag="rsum")
                nc.tensor.matmul(s_ps[:], ones_col[:], alpha[:], start=True, stop=True)
                # ln(s~) accumulated (off critical chain)
                ln_s = rpool.tile([1, NB], f32, tag="lns")
                nc.scalar.activation(ln_s[:], s_ps[:], AF.Ln)
                nc.vector.tensor_add(acc[:], acc[:], ln_s[:])
                # 1/s~ broadcast to all partitions via rank-1 matmul
                rs = rpool.tile([1, NB], f32, tag="rs")
                nc.vector.reciprocal(rs[:], s_ps[:])
                rs_h = rpool.tile([1, NB], bf16, tag="rsh")
                nc.vector.tensor_copy(rs_h[:], rs[:])
                bc_ps = bpsum.tile([S, NB], f32, tag="bc")
                nc.tensor.matmul(bc_ps[:], ones_row[:], rs_h[:], start=True, stop=True)
                # fold into the Ehat slice consumed at t + REN_DELTA
                t_app = t + REN_DELTA
                e_ren = rpool.tile([S, NB], bf16, tag="eren")
                nc.vector.tensor_mul(e_ren[:], ehat_v[:, t_app, :], bc_ps[:])
                pending[t_app] = e_ren[:]
                n_ren += 1

        # ---------------- finalization ----------------
        fin_ps = rpsum.tile([1, NB], f32, tag="rsum")
        nc.tensor.matmul(fin_ps[:], ones_col[:], alpha[:], start=True, stop=True)
        ln_fin = singles.tile([1, NB], f32)
        nc.scalar.activation(ln_fin[:], fin_ps[:], AF.Ln)
        nc.vector.tensor_add(acc[:], acc[:], ln_fin[:])
        nc.vector.tensor_scalar(
            acc[:], acc[:], ln_sumi[:], None, op0=OP.subtract
        )
        nc.vector.tensor_scalar(
            acc[:], acc[:], float(t_len * math.log(32.0)), None, op0=OP.subtract
        )
        nc.sync.dma_start(out, acc[:])


def _build(t_len=T):
    import concourse.tile as tile
    from concourse import bacc, mybir

    f32 = mybir.dt.float32
    bf16 = mybir.dt.bfloat16

    nc = bacc.Bacc("TRN2", target_bir_lowering=False, debug=False)
    xT_t = nc.dram_tensor("xT", [ALPH, t_len * NB], bf16, kind="ExternalInput")
    aL_t = nc.dram_tensor("A_logits", [S, S], f32, kind="ExternalInput")
    bL_t = nc.dram_tensor("B_logits", [ALPH, S], f32, kind="ExternalInput")
    iL_t = nc.dram_tensor("I_logits", [S], f32, kind="ExternalInput")
    out_t = nc.dram_tensor("loglik", [NB], f32, kind="ExternalOutput")

    with tile.TileContext(nc) as tc:
        _kernel_body(tc, xT_t.ap(), aL_t.ap(), bL_t.ap(), iL_t.ap(), out_t.ap(), t_len)
    nc.compile()
    return nc


def _shard_inputs(inputs, A_logits, B_logits, I_logits, t_len=T):
    import ml_dtypes

    in_maps = []
    for c in range(NC):
        xc = inputs[c * NB : (c + 1) * NB, :t_len]          # [NB, t, 32]
        xTc = np.ascontiguousarray(
            xc.transpose(2, 1, 0).reshape(ALPH, t_len * NB)
        ).astype(ml_dtypes.bfloat16)
        in_maps.append(
            {
                "xT": xTc,
                "A_logits": np.ascontiguousarray(A_logits, dtype=np.float32),
                "B_logits": np.ascontiguousarray(B_logits, dtype=np.float32),
                "I_logits": np.ascontiguousarray(I_logits, dtype=np.float32),
            }
        )
    return in_maps


def kernel(inputs, A_logits, B_logits, I_logits):
    from concourse.bass_utils import run_bass_kernel_spmd

    global _COMPILED
    if _COMPILED is None:
        _COMPILED = _build()

    in_maps = _shard_inputs(inputs, A_logits, B_logits, I_logits)
    res = run_bass_kernel_spmd(_COMPILED, in_maps, list(range(NC)))
    out = np.concatenate([res.results[c]["loglik"] for c in range(NC)])
    return out.astype(np.float32)


# revision 4
# speedup vs baseline: 1.0241x; 1.0241x over previous
"""Trainium2 Bass kernel for nn_CgpHmmLayer (HMM forward-algorithm log-likelihood).

Problem: batch=64 one-hot sequences [64, 4096, 32], softmax-parameterized HMM
with 128 states; output loglik [64].

Sharding: data-parallel over batch across 8 NeuronCores (8 sequences/core),
A/B/I replicated. No collectives needed.

Device algorithm (per core, states on partitions):
  A   = softmax(A_logits, rows)            (bf16 stationary for the scan matmul)
  expB = exp(B_logits);  r32[s] = 32 / sum_a expB[a,s]
  Ehat^T[s, (t,b)] = r32[s] * (expB^T @ X^T)   (bf16, 8MB in SBUF)
      -- the emission einsum, with a constant 32x rescale folded in so the
         running forward variable stays O(1) and renorms can be sparse.
  alpha_0 = expI * Ehat_0
  step t:  alpha = (A^T @ alpha) * Ehat_t      (PE matmul -> DVE multiply)
  every 64 steps (tick t0=64k+48): s~_b = colsum(alpha_t0) via ones-matmul;
      1/s~ broadcast via rank-1 matmul and folded into the Ehat slice consumed
      at t0+16 (off the critical serial chain); ln(s~) accumulated.
  loglik = ln(colsum(alpha_T)) + sum ln(s~) - T*ln(32) - ln(sum expI)

The one-hot input is shipped pre-transposed as bf16 [32, T*8] per core (pure
layout marshalling; 0/1 are exact in bf16) so the emission matmul's contraction
dim (alphabet) lands on partitions without any on-device transpose.
"""
import math
from contextlib import ExitStack

import numpy as np

B, T, ALPH, S = 64, 4096, 32, 128
NC = 8
NB = B // NC  # sequences per core

REN_PERIOD = 256
REN_TICK = 224   # renorm ticks t0 = k*256 + 224
REN_DELTA = 32   # applied at t0 + 32
# Emission chunks sized so one matmul (~219ns) and one scale op (~196ns) fit
# inside the scan step's PE/DVE idle windows (~258ns each) — the emission
# precompute then rides along with the scan at zero wall-clock cost.
ECHUNK = 128     # = 16 time steps * NB columns

_COMPILED = None


def _kernel_body(tc, xT, aL, bL, iL, out, t_len):
    import concourse.bass as bass
    from concourse import mybir

    nc = tc.nc
    f32 = mybir.dt.float32
    bf16 = mybir.dt.bfloat16
    AX = mybir.AxisListType
    OP = mybir.AluOpType
    AF = mybir.ActivationFunctionType

    n_echunks = (t_len * NB) // ECHUNK

    with ExitStack() as ctx:
        singles = ctx.enter_context(tc.tile_pool(name="singles", bufs=1))
        epsum = ctx.enter_context(tc.tile_pool(name="epsum", bufs=2, space="PSUM"))
        spsum = ctx.enter_context(tc.tile_pool(name="spsum", bufs=2, space="PSUM"))
        rpsum = ctx.enter_context(tc.tile_pool(name="rpsum", bufs=1, space="PSUM"))
        bpsum = ctx.enter_context(tc.tile_pool(name="bpsum", bufs=1, space="PSUM"))
        apool = ctx.enter_context(tc.tile_pool(name="apool", bufs=3))
        rpool = ctx.enter_context(tc.tile_pool(name="rpool", bufs=2))

        # ---------------- parameter prep ----------------
        aL_sb = singles.tile([S, S], f32)
        nc.sync.dma_start(aL_sb[:], aL)
        bN_sb = singles.tile([ALPH, S], f32)
        nc.sync.dma_start(bN_sb[:], bL)
        iL_sb = singles.tile([S, 1], f32)
        nc.sync.dma_start(
            iL_sb[:], bass.AP(tensor=iL.tensor, offset=0, ap=[[1, S], [S, 1]])
        )

        # A = softmax(rows of A_logits), stored bf16 (scan stationary operand)
        rowmax = singles.tile([S, 1], f32)
        nc.vector.tensor_reduce(rowmax[:], aL_sb[:], axis=AX.X, op=OP.max)
        negmax = singles.tile([S, 1], f32)
        nc.vector.tensor_scalar_mul(negmax[:], rowmax[:], -1.0)
        expA = singles.tile([S, S], f32)
        nc.scalar.activation(expA[:], aL_sb[:], AF.Exp, bias=negmax[:], scale=1.0)
        rowsum = singles.tile([S, 1], f32)
        nc.vector.tensor_reduce(rowsum[:], expA[:], axis=AX.X, op=OP.add)
        rrow = singles.tile([S, 1], f32)
        nc.vector.reciprocal(rrow[:], rowsum[:])
        A_sb = singles.tile([S, S], bf16)
        nc.vector.tensor_scalar_mul(A_sb[:], expA[:], rrow[:])

        # expB (column softmax handled via r32 scale folded into Ehat).
        # B_logits ~ N(0,1) so exp() without max-subtraction is safe.
        expB = singles.tile([ALPH, S], bf16)
        nc.scalar.activation(expB[:], bN_sb[:], AF.Exp)
        ones32 = singles.tile([ALPH, 1], bf16)
        nc.vector.memset(ones32[:], 1.0)
        bsum_ps = bpsum.tile([S, 1], f32, tag="bsum")
        nc.tensor.matmul(bsum_ps[:], expB[:], ones32[:], start=True, stop=True)
        r32 = singles.tile([S, 1], f32)
        nc.vector.reciprocal(r32[:], bsum_ps[:])
        nc.vector.tensor_scalar_mul(r32[:], r32[:], 32.0)

        # expI (fp32 for the alpha_0 scale; bf16 for the sum matmul)
        expI = singles.tile([S, 1], f32)
        nc.scalar.activation(expI[:], iL_sb[:], AF.Exp)
        expI_h = singles.tile([S, 1], bf16)
        nc.vector.tensor_copy(expI_h[:], expI[:])

        ones_col = singles.tile([S, 1], bf16)
        nc.vector.memset(ones_col[:], 1.0)
        ones_row = singles.tile([1, S], bf16)
        nc.vector.memset(ones_row[:], 1.0)

        sumi_ps = bpsum.tile([1, 1], f32, tag="bsum")
        nc.tensor.matmul(sumi_ps[:], ones_col[:], expI_h[:], start=True, stop=True)
        ln_sumi = singles.tile([1, 1], f32)
        nc.scalar.activation(ln_sumi[:], sumi_ps[:], AF.Ln)

        # ---------------- emission precompute (pipelined into the scan) ----
        xT_sb = singles.tile([ALPH, t_len * NB], bf16)
        ndma = 8
        dma_w = (t_len * NB) // ndma
        for i in range(ndma):
            nc.sync.dma_start(
                xT_sb[:, i * dma_w : (i + 1) * dma_w], xT[:, i * dma_w : (i + 1) * dma_w]
            )

        ehat = singles.tile([S, t_len * NB], bf16)
        ehat_v = ehat[:].rearrange("s (t nb) -> s t nb", nb=NB)

        def emit_echunk(c):
            lo, hi = c * ECHUNK, (c + 1) * ECHUNK
            e_ps = epsum.tile([S, ECHUNK], f32, tag="eps")
            nc.tensor.matmul(e_ps[:], expB[:], xT_sb[:, lo:hi], start=True, stop=True)
            nc.vector.tensor_scalar_mul(ehat[:, lo:hi], e_ps[:], r32[:])

        emit_echunk(0)

        # ---------------- the scan ----------------
        acc = singles.tile([1, NB], f32)
        nc.vector.memset(acc[:], 0.0)

        alpha = apool.tile([S, NB], bf16, tag="alpha")
        nc.vector.tensor_scalar_mul(alpha[:], ehat_v[:, 0, :], expI[:])

        n_ren = 0
        pending = {}  # t_app -> bcast psum tile
        for t in range(1, t_len):
            mm_ps = spsum.tile([S, NB], f32, tag="mm")
            nc.tensor.matmul(mm_ps[:], A_sb[:], alpha[:], start=True, stop=True)

            if t in pending:
                e_in = pending.pop(t)
            else:
                e_in = ehat_v[:, t, :]
            alpha_new = apool.tile([S, NB], bf16, tag="alpha")
            nc.vector.tensor_mul(alpha_new[:], mm_ps[:], e_in)
            alpha = alpha_new

            # one emission chunk per step rides in the engines' idle windows;
            # chunk t is consumed at scan step 16*t so production stays ahead
            if t < n_echunks:
                emit_echunk(t)

            if t % REN_PERIOD == REN_TICK and t + REN_DELTA < t_len:
                # column sums of alpha via ones-matmul (cheap PE visit)
                s_ps = rpsum.tile([1, NB], f32, tag="rsum")
                nc.tensor.matmul(s_ps[:], ones_col[:], alpha[:], start=True, stop=True)
                # ln(s~) accumulated (off critical chain)
                ln_s = rpool.tile([1, NB], f32, tag="lns")
                nc.scalar.activation(ln_s[:], s_ps[:], AF.Ln)
                nc.vector.tensor_add(acc[:], acc[:], ln_s[:])
                # 1/s~ broadcast to all partitions via rank-1 matmul
                rs = rpool.tile([1, NB], f32, tag="rs")
                nc.vector.reciprocal(rs[:], s_ps[:])
                rs_h = rpool.tile([1, NB], bf16, tag="rsh")
                nc.vector.tensor_copy(rs_h[:], rs[:])
                bc_ps = bpsum.tile([S, NB], f32, tag="bc")
                nc.tensor.matmul(bc_ps[:], ones_row[:], rs_h[:], start=True, stop=True)
                # fold into the Ehat slice consumed at t + REN_DELTA
                t_app = t + REN_DELTA
                e_ren = rpool.tile([S, NB], bf16, tag="eren")
                nc.vector.tensor_mul(e_ren[:], ehat_v[:, t_app, :], bc_ps[:])
                pending[t_app] = e_ren[:]
                n_ren += 1

        # ---------------- finalization ----------------
        fin_ps = rpsum.tile([1, NB], f32, tag="rsum")
        nc.tensor.matmul(fin_ps[:], ones_col[:], alpha[:], start=True, stop=True)
        ln_fin = singles.tile([1, NB], f32)
        nc.scalar.activation(ln_fin[:], fin_ps[:], AF.Ln)
        nc.vector.tensor_add(acc[:], acc[:], ln_fin[:])
        nc.vector.tensor_scalar(
            acc[:], acc[:], ln_sumi[:], None, op0=OP.subtract
        )
        nc.vector.tensor_scalar(
            acc[:], acc[:], float(t_len * math.log(32.0)), None, op0=OP.subtract
        )
        nc.sync.dma_start(out, acc[:])


def _build(t_len=T):
    import concourse.tile as tile
    from concourse import bacc, mybir

    f32 = mybir.dt.float32
    bf16 = mybir.dt.bfloat16

    nc = bacc.Bacc("TRN2", target_bir_lowering=False, debug=False)
    xT_t = nc.dram_tensor("xT", [ALPH, t_len * NB], bf16, kind="ExternalInput")
    aL_t = nc.dram_tensor("A_logits", [S, S], f32, kind="ExternalInput")
    bL_t = nc.dram_tensor("B_logits", [ALPH, S], f32, kind="ExternalInput")
    iL_t = nc.dram_tensor("I_logits", [S], f32, kind="ExternalInput")
    out_t = nc.dram_tensor("loglik", [NB], f32, kind="ExternalOutput")

    with tile.TileContext(nc) as tc:
        _kernel_body(tc, xT_t.ap(), aL_t.ap(), bL_t.ap(), iL_t.ap(), out_t.ap(), t_len)
    nc.compile()
    return nc


def _shard_inputs(inputs, A_logits, B_logits, I_logits, t_len=T):
    import ml_dtypes

    in_maps = []
    for c in range(NC):
        xc = inputs[c * NB : (c + 1) * NB, :t_len]          # [NB, t, 32]
        xTc = np.ascontiguousarray(
            xc.transpose(2, 1, 0).reshape(ALPH, t_len * NB)
        ).astype(ml_dtypes.bfloat16)
        in_maps.append(
            {
                "xT": xTc,
                "A_logits": np.ascontiguousarray(A_logits, dtype=np.float32),
                "B_logits": np.ascontiguousarray(B_logits, dtype=np.float32),
                "I_logits": np.ascontiguousarray(I_logits, dtype=np.float32),
            }
        )
    return in_maps


def kernel(inputs, A_logits, B_logits, I_logits):
    from concourse.bass_utils import run_bass_kernel_spmd

    global _COMPILED
    if _COMPILED is None:
        _COMPILED = _build()

    in_maps = _shard_inputs(inputs, A_logits, B_logits, I_logits)
    res = run_bass_kernel_spmd(_COMPILED, in_maps, list(range(NC)))
    out = np.concatenate([res.results[c]["loglik"] for c in range(NC)])
    return out.astype(np.float32)


# revision 8
# speedup vs baseline: 1.8611x; 1.8174x over previous
"""Trainium2 Bass kernel for nn_CgpHmmLayer (HMM forward-algorithm log-likelihood).

Problem: batch=64 one-hot sequences [64, 4096, 32], softmax-parameterized HMM
with 128 states; output loglik [64].

Sharding: data-parallel over batch across 8 NeuronCores (8 sequences/core),
A/B/I replicated. No collectives needed.

Device algorithm (per core, states on partitions):
  A   = softmax(A_logits, rows)            (bf16 stationary for the scan matmul)
  expB = exp(B_logits);  r32[s] = 32 / sum_a expB[a,s]
  Ehat^T[s, (t,b)] = r32[s] * (expB^T @ X^T)   (bf16, 8MB in SBUF)
      -- the emission einsum, with a constant 32x rescale folded in so the
         running forward variable stays O(1) and renorms can be sparse.
  alpha_0 = expI * Ehat_0
  step t:  alpha = (A^T @ alpha) * Ehat_t      (PE matmul -> DVE multiply)
  every 64 steps (tick t0=64k+48): s~_b = colsum(alpha_t0) via ones-matmul;
      1/s~ broadcast via rank-1 matmul and folded into the Ehat slice consumed
      at t0+16 (off the critical serial chain); ln(s~) accumulated.
  loglik = ln(colsum(alpha_T)) + sum ln(s~) - T*ln(32) - ln(sum expI)

The one-hot input is shipped pre-transposed as bf16 [32, T*8] per core (pure
layout marshalling; 0/1 are exact in bf16) so the emission matmul's contraction
dim (alphabet) lands on partitions without any on-device transpose.
"""
import math
from contextlib import ExitStack

import numpy as np

B, T, ALPH, S = 64, 4096, 32, 128
NC = 8
NB = B // NC  # sequences per core

REN_PERIOD = 256
REN_TICK = 224   # renorm ticks t0 = k*256 + 224
REN_DELTA = 32   # applied at t0 + 32
# Emission chunks sized so one matmul (~219ns) and one scale op (~196ns) fit
# inside the scan step's PE/DVE idle windows (~258ns each) — the emission
# precompute then rides along with the scan at zero wall-clock cost.
ECHUNK = 128     # = 16 time steps * NB columns

_COMPILED = None


def _kernel_body(tc, xT, aL, bL, iL, out, t_len):
    import concourse.bass as bass
    from concourse import mybir

    nc = tc.nc
    f32 = mybir.dt.float32
    bf16 = mybir.dt.bfloat16
    AX = mybir.AxisListType
    OP = mybir.AluOpType
    AF = mybir.ActivationFunctionType

    n_echunks = (t_len * NB) // ECHUNK

    with ExitStack() as ctx:
        singles = ctx.enter_context(tc.tile_pool(name="singles", bufs=1))
        epsum = ctx.enter_context(tc.tile_pool(name="epsum", bufs=1, space="PSUM"))
        spsum = ctx.enter_context(tc.tile_pool(name="spsum", bufs=2, space="PSUM"))
        rpsum = ctx.enter_context(tc.tile_pool(name="rpsum", bufs=1, space="PSUM"))
        bpsum = ctx.enter_context(tc.tile_pool(name="bpsum", bufs=1, space="PSUM"))
        apool = ctx.enter_context(tc.tile_pool(name="apool", bufs=3))
        rpool = ctx.enter_context(tc.tile_pool(name="rpool", bufs=2))

        # ---------------- parameter prep ----------------
        aL_sb = singles.tile([S, S], f32)
        nc.sync.dma_start(aL_sb[:], aL)
        bN_sb = singles.tile([ALPH, S], f32)
        nc.sync.dma_start(bN_sb[:], bL)
        iL_sb = singles.tile([S, 1], f32)
        nc.sync.dma_start(
            iL_sb[:], bass.AP(tensor=iL.tensor, offset=0, ap=[[1, S], [S, 1]])
        )

        # A = softmax(rows of A_logits), stored bf16 (scan stationary operand)
        rowmax = singles.tile([S, 1], f32)
        nc.vector.tensor_reduce(rowmax[:], aL_sb[:], axis=AX.X, op=OP.max)
        negmax = singles.tile([S, 1], f32)
        nc.vector.tensor_scalar_mul(negmax[:], rowmax[:], -1.0)
        expA = singles.tile([S, S], f32)
        nc.scalar.activation(expA[:], aL_sb[:], AF.Exp, bias=negmax[:], scale=1.0)
        rowsum = singles.tile([S, 1], f32)
        nc.vector.tensor_reduce(rowsum[:], expA[:], axis=AX.X, op=OP.add)
        rrow = singles.tile([S, 1], f32)
        nc.vector.reciprocal(rrow[:], rowsum[:])
        A_sb = singles.tile([S, S], bf16)
        nc.vector.tensor_scalar_mul(A_sb[:], expA[:], rrow[:])

        # expB (column softmax handled via r32 scale folded into Ehat).
        # B_logits ~ N(0,1) so exp() without max-subtraction is safe.
        expB = singles.tile([ALPH, S], bf16)
        nc.scalar.activation(expB[:], bN_sb[:], AF.Exp)
        ones32 = singles.tile([ALPH, 1], bf16)
        nc.vector.memset(ones32[:], 1.0)
        bsum_ps = bpsum.tile([S, 1], f32, tag="bsum")
        nc.tensor.matmul(bsum_ps[:], expB[:], ones32[:], start=True, stop=True)
        r32 = singles.tile([S, 1], f32)
        nc.vector.reciprocal(r32[:], bsum_ps[:])
        nc.vector.tensor_scalar_mul(r32[:], r32[:], 32.0)

        # expI (fp32 for the alpha_0 scale; bf16 for the sum matmul)
        expI = singles.tile([S, 1], f32)
        nc.scalar.activation(expI[:], iL_sb[:], AF.Exp)
        expI_h = singles.tile([S, 1], bf16)
        nc.vector.tensor_copy(expI_h[:], expI[:])

        ones_col = singles.tile([S, 1], bf16)
        nc.vector.memset(ones_col[:], 1.0)
        ones_row = singles.tile([1, S], bf16)
        nc.vector.memset(ones_row[:], 1.0)

        sumi_ps = bpsum.tile([1, 1], f32, tag="bsum")
        nc.tensor.matmul(sumi_ps[:], ones_col[:], expI_h[:], start=True, stop=True)
        ln_sumi = singles.tile([1, 1], f32)
        nc.scalar.activation(ln_sumi[:], sumi_ps[:], AF.Ln)

        # ---------------- emission precompute (pipelined into the scan) ----
        xT_sb = singles.tile([ALPH, t_len * NB], bf16)
        ndma = 8
        dma_w = (t_len * NB) // ndma
        for i in range(ndma):
            nc.sync.dma_start(
                xT_sb[:, i * dma_w : (i + 1) * dma_w], xT[:, i * dma_w : (i + 1) * dma_w]
            )

        ehat = singles.tile([S, t_len * NB], bf16)
        ehat_v = ehat[:].rearrange("s (t nb) -> s t nb", nb=NB)

        def emit_echunk(c):
            lo, hi = c * ECHUNK, (c + 1) * ECHUNK
            e_ps = epsum.tile([S, ECHUNK], f32, tag="eps")
            nc.tensor.matmul(e_ps[:], expB[:], xT_sb[:, lo:hi], start=True, stop=True)
            nc.vector.tensor_scalar_mul(ehat[:, lo:hi], e_ps[:], r32[:])

        emit_echunk(0)
        emit_echunk(n_echunks - 1)

        # A^T for the backward chain: regular matmul A^T = lhsT.T @ I with
        # lhsT = A (identity built from two iotas; no transpose-mode needed)
        iot_f = singles.tile([S, S], mybir.dt.int32)
        nc.gpsimd.iota(iot_f[:], pattern=[[1, S]], base=0, channel_multiplier=0)
        iot_p = singles.tile([S, S], mybir.dt.int32)
        nc.gpsimd.iota(iot_p[:], pattern=[[0, S]], base=0, channel_multiplier=1)
        ident = singles.tile([S, S], bf16)
        nc.vector.tensor_tensor(ident[:], iot_f[:], iot_p[:], op=OP.is_equal)
        at_ps = epsum.tile([S, S], f32, tag="eps")
        nc.tensor.matmul(at_ps[:], A_sb[:], ident[:], start=True, stop=True)
        AT_sb = singles.tile([S, S], bf16)
        nc.vector.tensor_copy(AT_sb[:], at_ps[:])

        # ---------------- the scan: forward and backward chains meet in the
        # middle.  loglik = log(u_m^T alpha_m):  alpha runs t=0..MEET,
        # u_{t-1} = A (e_t * u_t) runs t=T-1..MEET+1 (u_{T-1}=1). The two
        # serial chains interleave on PE/DVE, halving the wall clock.
        acc = singles.tile([1, NB], f32)
        nc.vector.memset(acc[:], 0.0)

        MEET = t_len // 2 - 1
        nsteps = t_len - 1 - MEET  # backward step count

        alpha = apool.tile([S, NB], bf16, tag="alpha")
        nc.vector.tensor_scalar_mul(alpha[:], ehat_v[:, 0, :], expI[:])

        def renorm(src_sb, t_app, pend):
            # column sums via ones-matmul; ln(s~) accumulated; 1/s~ broadcast
            # via rank-1 matmul and folded into the Ehat slice used at t_app.
            s_ps = rpsum.tile([1, NB], f32, tag="rsum")
            nc.tensor.matmul(s_ps[:], ones_col[:], src_sb, start=True, stop=True)
            ln_s = rpool.tile([1, NB], f32, tag="lns")
            nc.scalar.activation(ln_s[:], s_ps[:], AF.Ln)
            nc.vector.tensor_add(acc[:], acc[:], ln_s[:])
            rs = rpool.tile([1, NB], f32, tag="rs")
            nc.vector.reciprocal(rs[:], s_ps[:])
            rs_h = rpool.tile([1, NB], bf16, tag="rsh")
            nc.vector.tensor_copy(rs_h[:], rs[:])
            bc_ps = bpsum.tile([S, NB], f32, tag="bc")
            nc.tensor.matmul(bc_ps[:], ones_row[:], rs_h[:], start=True, stop=True)
            e_ren = rpool.tile([S, NB], bf16, tag="eren")
            nc.vector.tensor_mul(e_ren[:], ehat_v[:, t_app, :], bc_ps[:])
            pend[t_app] = e_ren[:]

        pend_f, pend_b = {}, {}
        # backward step 0: w_{T-1} = e_{T-1} * 1 — feed the Ehat slice directly
        ub_ps = spsum.tile([S, NB], f32, tag="mmb")
        nc.tensor.matmul(ub_ps[:], AT_sb[:], ehat_v[:, t_len - 1, :], start=True, stop=True)

        for k in range(1, nsteps):
            t_f = k  # forward step index (1..MEET)
            t_b = (t_len - 1) - k  # backward emission index (T-2 .. MEET+1)

            if t_f <= MEET:
                mmf_ps = spsum.tile([S, NB], f32, tag="mmf")
                nc.tensor.matmul(mmf_ps[:], A_sb[:], alpha[:], start=True, stop=True)

            # backward: w_t = u_t * e_t  (u_t sits in the previous MM's psum)
            e_in_b = pend_b.pop(t_b, None)
            if e_in_b is None:
                e_in_b = ehat_v[:, t_b, :]
            w = apool.tile([S, NB], bf16, tag="wbwd")
            nc.vector.tensor_mul(w[:], ub_ps[:], e_in_b)

            if t_f <= MEET:
                e_in_f = pend_f.pop(t_f, None)
                if e_in_f is None:
                    e_in_f = ehat_v[:, t_f, :]
                alpha_new = apool.tile([S, NB], bf16, tag="alpha")
                nc.vector.tensor_mul(alpha_new[:], mmf_ps[:], e_in_f)
                alpha = alpha_new

            ub_ps = spsum.tile([S, NB], f32, tag="mmb")
            nc.tensor.matmul(ub_ps[:], AT_sb[:], w[:], start=True, stop=True)

            # emission chunk production: one (front, back) pair every 8 steps
            if k % 8 == 0 and 1 <= k // 8 <= (n_echunks // 2 - 1):
                emit_echunk(k // 8)
                emit_echunk(n_echunks - 1 - k // 8)

            if t_f % REN_PERIOD == REN_TICK and t_f + REN_DELTA <= MEET:
                renorm(alpha[:], t_f + REN_DELTA, pend_f)
            if k % REN_PERIOD == REN_TICK and t_b - REN_DELTA > MEET:
                renorm(w[:], t_b - REN_DELTA, pend_b)

        # ---------------- finalization: z = alpha_m * u_m, loglik pieces ----
        z = rpool.tile([S, NB], bf16, tag="zfin")
        nc.vector.tensor_mul(z[:], ub_ps[:], alpha[:])
        fin_ps = rpsum.tile([1, NB], f32, tag="rsum")
        nc.tensor.matmul(fin_ps[:], ones_col[:], z[:], start=True, stop=True)
        ln_fin = singles.tile([1, NB], f32)
        nc.scalar.activation(ln_fin[:], fin_ps[:], AF.Ln)
        nc.vector.tensor_add(acc[:], acc[:], ln_fin[:])
        nc.vector.tensor_scalar(
            acc[:], acc[:], ln_sumi[:], None, op0=OP.subtract
        )
        nc.vector.tensor_scalar(
            acc[:], acc[:], float(t_len * math.log(32.0)), None, op0=OP.subtract
        )
        nc.sync.dma_start(out, acc[:])


def _build(t_len=T):
    import concourse.tile as tile
    from concourse import bacc, mybir

    f32 = mybir.dt.float32
    bf16 = mybir.dt.bfloat16

    nc = bacc.Bacc("TRN2", target_bir_lowering=False, debug=False)
    xT_t = nc.dram_tensor("xT", [ALPH, t_len * NB], bf16, kind="ExternalInput")
    aL_t = nc.dram_tensor("A_logits", [S, S], f32, kind="ExternalInput")
    bL_t = nc.dram_tensor("B_logits", [ALPH, S], f32, kind="ExternalInput")
    iL_t = nc.dram_tensor("I_logits", [S], f32, kind="ExternalInput")
    out_t = nc.dram_tensor("loglik", [NB], f32, kind="ExternalOutput")

    with tile.TileContext(nc) as tc:
        _kernel_body(tc, xT_t.ap(), aL_t.ap(), bL_t.ap(), iL_t.ap(), out_t.ap(), t_len)
    nc.compile()
    return nc


def _shard_inputs(inputs, A_logits, B_logits, I_logits, t_len=T):
    import ml_dtypes

    in_maps = []
    for c in range(NC):
        xc = inputs[c * NB : (c + 1) * NB, :t_len]          # [NB, t, 32]
        xTc = np.ascontiguousarray(
            xc.transpose(2, 1, 0).reshape(ALPH, t_len * NB)
        ).astype(ml_dtypes.bfloat16)
        in_maps.append(
            {
                "xT": xTc,
                "A_logits": np.ascontiguousarray(A_logits, dtype=np.float32),
                "B_logits": np.ascontiguousarray(B_logits, dtype=np.float32),
                "I_logits": np.ascontiguousarray(I_logits, dtype=np.float32),
            }
        )
    return in_maps


def kernel(inputs, A_logits, B_logits, I_logits):
    from concourse.bass_utils import run_bass_kernel_spmd

    global _COMPILED
    if _COMPILED is None:
        _COMPILED = _build()

    in_maps = _shard_inputs(inputs, A_logits, B_logits, I_logits)
    res = run_bass_kernel_spmd(_COMPILED, in_maps, list(range(NC)))
    out = np.concatenate([res.results[c]["loglik"] for c in range(NC)])
    return out.astype(np.float32)
